# revision 1
# baseline (speedup 1.0000x reference)
"""Single-head causal attention (B=8, S=2048, D=1024, H=64).

Data-parallel over batch: each of the 8 NeuronCores computes one batch
element's full attention head.  Per core:

  qT/kT = (Wq|Wk)^T @ idx^T          -> [128, S]  (rows 0-63 qT, 64-127 kT)
  vT    = Wv^T @ idx^T               -> col-packed pairs of q-blocks
  v     = transpose(vT)              -> [S/128 x 128, 64] + ones column
  sT[k,q] = kT_tile^T @ qT           -> row-packed pairs (k on partitions)
  p = exp(sT / sqrt(D)) * tril_mask  (no max subtraction: |s| <= ~2)
  oT[65, q] += v_aug[k]^T @ p        -> rows 0-63 out, row 64 = sum(exp) = Z
  out[q, h] = transpose(oT)[:, :64] / Z[q]

All TensorE-facing tensors are bf16 (f32 PSUM accumulation); epilogue f32.
The host pre-shards idx[i] into a transposed, bf16, block-major layout so
each q-block's columns arrive as one fully-contiguous 1 MiB DMA.
"""

import sys

for _p in ("/opt/trn_rl_repo",):
    if _p not in sys.path:
        sys.path.insert(0, _p)

import numpy as np
import ml_dtypes

import concourse.bacc as bacc
import concourse.bass as bass
import concourse.mybir as mybir
from concourse import masks, tile
from concourse.bass_utils import run_bass_kernel_spmd

B, S, D, H = 8, 2048, 1024, 64
P = 128
QB = 512            # q-block width (one PSUM bank of f32)
NB = S // QB        # 4 q-blocks
KT = S // P         # 16 k-tiles
DT = D // P         # 8 d-tiles
SCALE = float(D) ** -0.5  # 1/32, exact in bf16/f32

BF16 = mybir.dt.bfloat16
F32 = mybir.dt.float32
AF = mybir.ActivationFunctionType

TRACE = False
LAST_RESULT = None


def enable_trace():
    """Register the NTFF profile hook that the agent image's antenv lacks,
    and neuter the artifact upload (no bucket in this container)."""
    global TRACE
    import types

    import antenv
    import concourse.bass_utils as bu

    if "antenv.axon_hooks" not in sys.modules:
        mod = types.ModuleType("antenv.axon_hooks")
        mod._hook = None
        mod.set_axon_ntff_profile_hook = lambda h: setattr(mod, "_hook", h)
        mod.get_axon_ntff_profile_hook = lambda: mod._hook
        sys.modules["antenv.axon_hooks"] = mod
        antenv.axon_hooks = mod
    from trn_agent_boot.trn_boot import _ntff_profile_via_ctypes

    sys.modules["antenv.axon_hooks"].set_axon_ntff_profile_hook(
        _ntff_profile_via_ctypes("/opt/axon/libaxon_pjrt.so")
    )
    bu.upload_artifacts = lambda tmpdir: tmpdir
    TRACE = True


def build_bass():
    nc = bacc.Bacc("TRN2", target_bir_lowering=False, debug=False, num_devices=B)
    # block-major swizzle: idxT_d[b, p, t*QB + s] = idx[b*QB + s, t*P + p]
    idxT_d = nc.declare_dram_parameter("idxT", [NB, P, DT * QB], BF16, isOutput=False)
    wqk_d = nc.declare_dram_parameter("wqk", [D, 2 * H], BF16, isOutput=False)
    wv_d = nc.declare_dram_parameter("wv", [D, H], BF16, isOutput=False)
    # out[p, t*H + h] = attention_out[t*128 + p, h]
    out_d = nc.declare_dram_parameter("out", [P, KT * H], F32, isOutput=True)

    with tile.TileContext(nc) as tc:
        with (
            tc.tile_pool(name="consts", bufs=1) as consts,
            tc.tile_pool(name="data", bufs=1) as data,
            tc.tile_pool(name="pp", bufs=8) as pp,
            tc.tile_pool(name="ep", bufs=2) as ep,
            tc.tile_pool(name="ps_mm", bufs=3, space="PSUM") as ps_mm,
            tc.tile_pool(name="ps_o", bufs=2, space="PSUM") as ps_o,
        ):
            ps_t = ps_o  # transposes share the ps_o slots (disjoint in time)

            # ---------------- warmup ----------------
            # dummy matmuls on garbage SBUF keep the PE HAM clock-gate warm
            # while the input DMAs stream in; results are never read
            junk = consts.tile([P, QB], BF16)
            nc.gpsimd.memset(junk[:], 0.0)
            wps = ps_mm.tile([P, 2 * QB], F32, tag="mm")
            for w in range(9):
                nc.tensor.matmul(
                    wps[:, 0:QB], junk[:, 0:P], junk[:], start=True, stop=True
                )

            # ---------------- constants ----------------
            # weights go on the scalar HWDGE queue: sync queue is reserved
            # for the big idxT chunks, gpsimd for SBUF<->SBUF shuffles
            wqk_sb = consts.tile([P, DT, 2 * H], BF16)
            nc.scalar.dma_start(wqk_sb[:], wqk_d.rearrange("(t p) m -> p t m", p=P))
            wv_sb = consts.tile([P, DT, H], BF16)
            nc.scalar.dma_start(wv_sb[:], wv_d.rearrange("(t p) m -> p t m", p=P))
            ident = consts.tile([P, P], BF16)
            masks.make_identity(nc, ident[:])
            ident32 = consts.tile([P, P], F32)
            masks.make_identity(nc, ident32[:])
            # mask[k, q] = 1.0 where q >= k (upper triangular incl diagonal)
            mask_sb = consts.tile([P, P], BF16)
            masks.make_upper_triangular(nc, mask_sb[:], val=1.0, diag=True)
            # identity on partitions 64-127 (for transposing hi-half vT)
            identh = consts.tile([P, H], BF16)
            nc.gpsimd.memset(identh[:], 0.0)
            nc.gpsimd.affine_select(
                out=identh[:],
                in_=identh[:],
                compare_op=mybir.AluOpType.not_equal,
                fill=1.0,
                base=-H,
                pattern=[[-1, H]],
                channel_multiplier=1,
            )

            # ---------------- working tiles ----------------
            qkT_sb = data.tile([P, S], BF16)   # rows 0-63 qT, rows 64-127 kT
            qkT2_sb = data.tile([P, S], BF16)  # swapped: rows 0-63 kT, 64-127 qT
            vT_sb = data.tile([P, S // 2], BF16)  # even blocks rows 0-63, odd 64-127
            v_sb = data.tile([P, KT, H + 1], BF16)  # [k, 64 v | 1.0]
            idxT_sb = data.tile([P, DT, S], BF16)
            out_sb = data.tile([P, KT, H], F32)

            # ones column of v_aug (flash-style Z accumulator row)
            nc.vector.memset(v_sb[:, :, H : H + 1].rearrange("p t o -> p (t o)"), 1.0)

            def load_idxT_block(b, splits=1):
                # fully-contiguous chunks: all d-tiles for block b; block 0 is
                # split so the first projection matmuls can start earlier
                cols = slice(b * QB, (b + 1) * QB)
                src = idxT_d[b].rearrange("p (t s) -> p t s", t=DT)
                step = DT // splits
                for u in range(splits):
                    ts = slice(u * step, (u + 1) * step)
                    nc.sync.dma_start(idxT_sb[:, ts, cols], src[:, ts, :])

            def proj_block(b):
                cols = slice(b * QB, (b + 1) * QB)
                ps = ps_mm.tile([P, 2 * QB], F32, tag="mm")
                for t in range(DT):
                    nc.tensor.matmul(
                        ps[:, 0:QB],
                        wqk_sb[:, t, :],
                        idxT_sb[:, t, cols],
                        start=(t == 0),
                        stop=(t == DT - 1),
                    )
                nc.vector.tensor_copy(qkT_sb[:, cols], ps[:, 0:QB])
                # scores row-packing needs kT on partitions 0-63 (pair slot A)
                # and qT on partitions 64-127 (pair slot B): SBUF->SBUF swap
                # copies on the gpsimd SWDGE ring
                nc.gpsimd.dma_start(qkT2_sb[0:H, cols], qkT_sb[H:P, cols])
                nc.gpsimd.dma_start(qkT2_sb[H:P, cols], qkT_sb[0:H, cols])

            def proj_v_pair(b):
                # col-packed pair: vT(b) -> psum parts 0-63, vT(b+1) -> 64-127
                colsa = slice(b * QB, (b + 1) * QB)
                colsb = slice((b + 1) * QB, (b + 2) * QB)
                hcols = slice(b * QB // 2, (b + 2) * QB // 2)
                ps = ps_mm.tile([P, 2 * QB], F32, tag="mm")
                for t in range(DT):
                    nc.tensor.matmul(
                        ps[0:H, 0:QB],
                        wv_sb[:, t, :],
                        idxT_sb[:, t, colsa],
                        start=(t == 0),
                        stop=(t == DT - 1),
                    )
                    nc.tensor.matmul(
                        ps[H:P, 0:QB],
                        wv_sb[:, t, :],
                        idxT_sb[:, t, colsb],
                        start=(t == 0),
                        stop=(t == DT - 1),
                        tile_position=(0, H),
                    )
                nc.vector.tensor_copy(vT_sb[:, hcols], ps[:, 0:QB])
                # v natural layout [k, 64] via PE transpose of vT halves;
                # 4 transposes share one PSUM bank (start only on the first),
                # evacuated by a single DVE copy
                for g in (0, 4):
                    j0 = 4 * b + g
                    pst = ps_t.tile([P, 4, H], BF16, tag="po", name=f"vt_{b}_{g}")
                    for u in range(4):
                        j = j0 + u
                        jj = (j - 4 * b) % 4 + 2 * b  # column tile within vT_sb
                        hi = j >= 4 * b + 4
                        src = (
                            vT_sb[H:P, jj * P : (jj + 1) * P]
                            if hi
                            else vT_sb[0:H, jj * P : (jj + 1) * P]
                        )
                        nc.tensor.matmul(
                            pst[:, u, :],
                            src,
                            identh[H:P, :] if hi else ident[:H, :H],
                            is_transpose=True,
                            start=(u == 0),
                            stop=(u == 3),
                            skip_group_check=True,
                        )
                    nc.vector.tensor_copy(v_sb[:, j0 : j0 + 4, 0:H], pst[:])

            def attention_pairs(qb, po):
                """Emit one (qb, pair) step at a time via a generator so two
                q-blocks' streams can be interleaved."""
                last_i = 4 * qb + 3
                for m in range((last_i + 1) // 2):
                    ia, ib = 2 * m, 2 * m + 1
                    offa = max(0, 128 * ia - QB * qb)
                    offb = max(0, 128 * ib - QB * qb)
                    # row-packed score pair: A on array rows 0-63,
                    # B on rows 64-127, concurrent on the PE
                    ps = ps_mm.tile([P, 2 * QB], F32, tag="mm")
                    nc.tensor.matmul(
                        ps[:, offa:QB],
                        qkT2_sb[0:H, ia * P : (ia + 1) * P],
                        qkT_sb[0:H, qb * QB + offa : (qb + 1) * QB],
                        start=True,
                        stop=True,
                    )
                    nc.tensor.matmul(
                        ps[:, QB + offb : 2 * QB],
                        qkT_sb[H:P, ib * P : (ib + 1) * P],
                        qkT2_sb[H:P, qb * QB + offb : (qb + 1) * QB],
                        start=True,
                        stop=True,
                    )
                    p_sb = pp.tile([P, 2 * QB], BF16, tag="p")
                    nc.scalar.activation(
                        p_sb[:, offa:], ps[:, offa:], AF.Exp, scale=SCALE
                    )
                    for i, off, base in ((ia, offa, 0), (ib, offb, QB)):
                        if i >= 4 * qb:  # diagonal tile: mask [128,128] block
                            cl = slice(base + off, base + off + P)
                            nc.vector.tensor_mul(p_sb[:, cl], p_sb[:, cl], mask_sb[:])
                    nc.tensor.matmul(
                        po[:, offa:],
                        v_sb[:, ia, :],
                        p_sb[:, offa:QB],
                        start=(ia == 0),
                        stop=False,
                        skip_group_check=True,
                    )
                    nc.tensor.matmul(
                        po[:, offb:],
                        v_sb[:, ib, :],
                        p_sb[:, QB + offb : 2 * QB],
                        start=False,
                        stop=(ib == last_i),
                        skip_group_check=True,
                    )
                    yield

            def attention_epilogue(qb, po):
                # transpose oT back to [q, 65], divide by Z, store; all 4
                # transposes share one PSUM bank
                oT_sb = ep.tile([H + 1, QB], F32, tag="ot")
                nc.vector.tensor_copy(oT_sb[:], po[:])
                pst = ps_t.tile([P, NB, H + 2], F32, tag="po", name=f"ot_{qb}")
                for c in range(NB):
                    nc.tensor.matmul(
                        pst[:, c, 0 : H + 1],
                        oT_sb[:, c * P : (c + 1) * P],
                        ident32[: H + 1, : H + 1],
                        is_transpose=True,
                        start=(c == 0),
                        stop=(c == NB - 1),
                        skip_group_check=True,
                    )
                rec = ep.tile([P, NB], F32, tag="rec")
                nc.vector.reciprocal(rec[:], pst[:, :, H])
                for c in range(NB):
                    j = 4 * qb + c
                    nc.vector.tensor_scalar_mul(
                        out_sb[:, j, :], pst[:, c, 0:H], rec[:, c : c + 1]
                    )
                nc.sync.dma_start(
                    out_d[:, qb * NB * H : (qb + 1) * NB * H],
                    out_sb[:, 4 * qb : 4 * qb + 4, :].rearrange("p t h -> p (t h)"),
                )

            def attention_blocks(qbs):
                pos = {
                    qb: ps_o.tile([H + 1, QB], F32, tag="po", name=f"po_{qb}")
                    for qb in qbs
                }
                gens = [(qb, attention_pairs(qb, pos[qb])) for qb in qbs]
                while gens:
                    nxt = []
                    for qb, g in gens:
                        try:
                            next(g)
                            nxt.append((qb, g))
                        except StopIteration:
                            attention_epilogue(qb, pos[qb])
                    gens = nxt

            load_idxT_block(0, splits=4)
            for b in range(1, NB):
                load_idxT_block(b)
            proj_block(0)
            proj_block(1)
            proj_v_pair(0)
            attention_blocks([0, 1])
            proj_block(2)
            proj_block(3)
            proj_v_pair(2)
            attention_blocks([2, 3])
    nc.compile()
    return nc


_NC = None


def _get_nc():
    global _NC
    if _NC is None:
        _NC = build_bass()
    return _NC


def kernel(idx, Wk, Wq, Wv):
    global LAST_RESULT
    idx = np.asarray(idx, dtype=np.float32)
    Wk = np.asarray(Wk, dtype=np.float32)
    Wq = np.asarray(Wq, dtype=np.float32)
    Wv = np.asarray(Wv, dtype=np.float32)

    wqk = np.concatenate([Wq, Wk], axis=1).astype(ml_dtypes.bfloat16)
    wv = Wv.astype(ml_dtypes.bfloat16)
    in_maps = []
    for i in range(B):
        idxT = np.ascontiguousarray(idx[i].T).astype(ml_dtypes.bfloat16)  # [D, S]
        # block-major swizzle: [b, p, t*QB + s] = idxT[t*P + p, b*QB + s]
        blk = np.ascontiguousarray(
            idxT.reshape(DT, P, NB, QB).transpose(2, 1, 0, 3).reshape(NB, P, DT * QB)
        )
        in_maps.append({"idxT": blk, "wqk": wqk, "wv": wv})

    res = run_bass_kernel_spmd(_get_nc(), in_maps, core_ids=list(range(B)), trace=TRACE)
    LAST_RESULT = res

    out = np.empty((B, S, H), dtype=np.float32)
    for i in range(B):
        o = np.asarray(res.results[i]["out"], dtype=np.float32)  # [P, KT*H]
        out[i] = o.reshape(P, KT, H).transpose(1, 0, 2).reshape(S, H)
    return out


if __name__ == "__main__":
    rng = np.random.default_rng(0)
    idx = rng.standard_normal((B, S, D), dtype=np.float32)
    Wk = rng.standard_normal((D, H), dtype=np.float32) / np.sqrt(D)
    Wq = rng.standard_normal((D, H), dtype=np.float32) / np.sqrt(D)
    Wv = rng.standard_normal((D, H), dtype=np.float32) / np.sqrt(D)
    o = kernel(idx=idx, Wk=Wk, Wq=Wq, Wv=Wv)
    print(o.shape, o.dtype, np.abs(o).mean())

